# revision 8
# baseline (speedup 1.0000x reference)
"""Multi-head attention (B=2, S=2048, D=1024, H=16, HD=64) on 8 TRN2 cores.

Sharding: batch x head-groups -> 8 shards (core c: batch c//4, heads
4*(c%4)..4*(c%4)+3). Each core projects its batch's q/k/v against its head
slice of Wq/Wk/Wv, computes full SxS attention for its 4 heads, and the
partial output projection for its head slice of Wo. Host sums the partials
per batch (+bo) and reassembles attn_weights.

On-device layout is "transposed": activations are kept head-dim-major
([hd, S]) so every matmul contracts over the partition axis:
  scoresT[Sk,Sq] = KT_h.T-tiles @ QT_h     (K=64 per head)
  softmax along Sk (partition axis): row sums via ones-matmul on PE,
  reciprocal + scale on DVE; exp on ACT (scale=1/sqrt(HD) folded in).
  outT[hd,Sq] = V-tiles(lhsT) @ wT-tiles   (K=Sk)
attn_weights are written to DRAM in [Sk,Sq] layout and transposed on host.
Matmul operands are bf16 (PE streams 1 col/cycle; fp32 is 4x slower),
accumulation is fp32 in PSUM.
"""
from contextlib import ExitStack

import numpy as np
import ml_dtypes

import concourse.bass as bass
import concourse.mybir as mybir
import concourse.tile as tile

B, S, D, H, HD = 2, 2048, 1024, 16, 64
N_CORES = 8
CORES_PER_B = N_CORES // B          # 4
HPC = H // CORES_PER_B              # heads per core = 4
SCALE = HD ** -0.5

BF16 = mybir.dt.bfloat16
F32 = mybir.dt.float32
AF = mybir.ActivationFunctionType

_bf16 = ml_dtypes.bfloat16


# --- workaround: this walrus build rejects >1 sync wait per instruction ---
_wsplit_ctr = [0]


def _split_waits(nc):
    n = 0
    for fn in nc.m.functions:
        for bb in fn.blocks:
            lst = bb.instructions
            i = 0
            while i < len(lst):
                inst = lst[i]
                si = inst.sync_info
                waits = list(si.on_wait) if si and si.on_wait else []
                if len(waits) > 1:
                    inst.sync_info = mybir.SyncInfo(
                        on_wait=[waits[-1]], on_update=list(si.on_update or [])
                    )
                    for w in waits[:-1]:
                        _wsplit_ctr[0] += 1
                        nop = mybir.InstNoOp(
                            name=f"Wsplit-{_wsplit_ctr[0]}", ins=[], outs=[]
                        )
                        nop.engine = inst.engine
                        nop.sync_info = mybir.SyncInfo(on_wait=[w], on_update=[])
                        nc.register_instruction(nop)
                        lst.insert(i, nop)
                        i += 1
                        n += 1
                i += 1
    return n


def build_nc(rep: int = 1, s: int = S, d: int = D, hpc: int = HPC):
    """Per-core kernel. rep>1 statically unrolls the body (for timing)."""
    KD = d // 128          # contraction tiles of the projections
    SK = s // 128          # Sk tiles (and Sq tiles of the output projection)
    NJ = s // 512          # Sq blocks of 512
    MQ = hpc * HD // 128   # 128-partition tiles of the per-core head dims
    HC = hpc * HD          # head-dim columns per core

    nc = bass.Bass("TRN2")
    qT = nc.dram_tensor("qT", [d, s], BF16, kind="ExternalInput")
    kT = nc.dram_tensor("kT", [d, s], BF16, kind="ExternalInput")
    vT = nc.dram_tensor("vT", [d, s], BF16, kind="ExternalInput")
    wq = nc.dram_tensor("wq", [d, HC], BF16, kind="ExternalInput")
    wk = nc.dram_tensor("wk", [d, HC], BF16, kind="ExternalInput")
    wv = nc.dram_tensor("wv", [d, HC], BF16, kind="ExternalInput")
    wo = nc.dram_tensor("wo", [HC, d], BF16, kind="ExternalInput")
    bq = nc.dram_tensor("bq", [HC, 1], F32, kind="ExternalInput")
    bk = nc.dram_tensor("bk", [HC, 1], F32, kind="ExternalInput")
    bv = nc.dram_tensor("bv", [HC, 1], F32, kind="ExternalInput")
    out_w = nc.dram_tensor("out_w", [hpc, s, s], F32, kind="ExternalOutput")
    out_p = nc.dram_tensor("out_p", [s, d], F32, kind="ExternalOutput")

    with tile.TileContext(nc) as tc, ExitStack() as ctx:
        const = ctx.enter_context(tc.tile_pool(name="const", bufs=1))
        persist = ctx.enter_context(tc.tile_pool(name="persist", bufs=1))
        xt_pool = ctx.enter_context(tc.tile_pool(name="xt", bufs=1))
        w_pool = ctx.enter_context(tc.tile_pool(name="w", bufs=2))
        et_pool = ctx.enter_context(tc.tile_pool(name="et", bufs=2))
        rc_pool = ctx.enter_context(tc.tile_pool(name="rc", bufs=2))
        ob_pool = ctx.enter_context(tc.tile_pool(name="ob", bufs=2))
        ps_s = ctx.enter_context(tc.tile_pool(name="ps_s", bufs=2, space="PSUM"))
        ps_w = ctx.enter_context(tc.tile_pool(name="ps_w", bufs=4, space="PSUM"))

        ones = const.tile([128, 128], BF16)
        nc.vector.memset(ones[:], 1.0)

        bq_sb = const.tile([128, MQ, 1], F32, tag="bq")
        bk_sb = const.tile([128, MQ, 1], F32, tag="bk")
        bv_sb = const.tile([64, hpc, 1], F32, tag="bv")
        nc.sync.dma_start(bq_sb[:], bq.rearrange("(m p) one -> p m one", p=128))
        nc.sync.dma_start(bk_sb[:], bk.rearrange("(m p) one -> p m one", p=128))
        nc.sync.dma_start(bv_sb[:], bv.rearrange("(h p) one -> p h one", p=64))

        # persistent activations
        QT = [persist.tile([128, s], BF16, tag=f"QT{m}", name=f"QT{m}") for m in range(MQ)]
        KT = [persist.tile([128, s], BF16, tag=f"KT{m}", name=f"KT{m}") for m in range(MQ)]
        V = persist.tile([128, SK, HC], BF16, tag="V")
        aoT = [persist.tile([64, s], BF16, tag=f"aoT{h}", name=f"aoT{h}") for h in range(hpc)]
        wo_sb = [persist.tile([64, d], BF16, tag=f"wo{h}", name=f"wo{h}") for h in range(hpc)]
        for h in range(hpc):
            nc.sync.dma_start(wo_sb[h][:], wo[h * HD:(h + 1) * HD, :])

        def proj_qk(xdram, wdram, bias_sb, OT):
            xt = xt_pool.tile([128, KD, s], BF16, tag="xt")
            nc.sync.dma_start(xt[:], xdram.rearrange("(k p) s -> p k s", p=128))
            wsb = w_pool.tile([128, KD, HC], BF16, tag="w")
            nc.sync.dma_start(wsb[:], wdram.rearrange("(k p) c -> p k c", p=128))
            for m in range(MQ):
                for n in range(s // 512):
                    ps = ps_w.tile([128, 512], F32, tag="ps")
                    for kk in range(KD):
                        nc.tensor.matmul(
                            ps[:],
                            wsb[:, kk, m * 128:(m + 1) * 128],
                            xt[:, kk, n * 512:(n + 1) * 512],
                            start=(kk == 0), stop=(kk == KD - 1),
                        )
                    nc.vector.tensor_scalar_add(
                        OT[m][:, n * 512:(n + 1) * 512], ps[:], bias_sb[:, m, :]
                    )

        def proj_v(xdram, wdram):
            xt = xt_pool.tile([128, KD, s], BF16, tag="xt")
            nc.sync.dma_start(xt[:], xdram.rearrange("(k p) s -> p k s", p=128))
            wsb = w_pool.tile([128, KD, HC], BF16, tag="w")
            nc.sync.dma_start(wsb[:], wdram.rearrange("(k p) c -> p k c", p=128))
            for m in range(SK):
                ps = ps_w.tile([128, HC], F32, tag="ps")
                for kk in range(KD):
                    nc.tensor.matmul(
                        ps[:],
                        xt[:, kk, m * 128:(m + 1) * 128],
                        wsb[:, kk, :],
                        start=(kk == 0), stop=(kk == KD - 1),
                    )
                nc.vector.tensor_copy(V[:, m, :], ps[:])

        def attn_block(h, j):
            mq, off = h // 2, 64 * (h % 2)
            KTh = KT[mq][off:off + 64, :]
            QTh = QT[mq][off:off + 64, :]
            et = et_pool.tile([128, SK, 512], BF16, tag="et")
            sums = ps_w.tile([128, 512], F32, tag="ps")
            for ii in range(SK // 2):
                ps = ps_s.tile([128, 2, 512], F32, tag="ps_s")
                for half in (0, 1):
                    i = 2 * ii + half
                    nc.tensor.matmul(
                        ps[:, half, :],
                        KTh[:, i * 128:(i + 1) * 128],
                        QTh[:, j * 512:(j + 1) * 512],
                        start=True, stop=True,
                    )
                nc.scalar.activation(
                    et[:, 2 * ii:2 * ii + 2, :], ps[:], AF.Exp, scale=SCALE
                )
                nc.tensor.matmul(sums[:], ones[:], et[:, 2 * ii, :],
                                 start=(ii == 0), stop=False)
                nc.tensor.matmul(sums[:], ones[:], et[:, 2 * ii + 1, :],
                                 start=False, stop=(ii == SK // 2 - 1))
            recip = rc_pool.tile([128, 512], BF16, tag="rc")
            with nc.allow_low_precision(reason="bf16 softmax normalizer"):
                nc.vector.reciprocal(recip[:], sums[:])
            for i in range(SK):
                nc.vector.tensor_mul(et[:, i, :], et[:, i, :], recip[:])
            nc.gpsimd.dma_start(
                out=out_w[h][:, j * 512:(j + 1) * 512]
                .rearrange("(i p) f -> p i f", p=128),
                in_=et[:],
            )
            ps_av = ps_w.tile([64, 512], F32, tag="ps")
            for i in range(SK):
                nc.tensor.matmul(
                    ps_av[:], V[:, i, HD * h:HD * (h + 1)], et[:, i, :],
                    start=(i == 0), stop=(i == SK - 1),
                )
            nc.vector.tensor_scalar_add(
                aoT[h][:, j * 512:(j + 1) * 512], ps_av[:], bv_sb[:, h, :]
            )

        def out_proj():
            CH = min(512, d)
            for m in range(SK):
                osb = ob_pool.tile([128, d], F32, tag="ob")
                for n in range(d // CH):
                    ps = ps_w.tile([128, CH], F32, tag="ps")
                    for h in range(hpc):
                        nc.tensor.matmul(
                            ps[:],
                            aoT[h][:, m * 128:(m + 1) * 128],
                            wo_sb[h][:, n * CH:(n + 1) * CH],
                            start=(h == 0), stop=(h == hpc - 1),
                        )
                    nc.vector.tensor_copy(osb[:, n * CH:(n + 1) * CH], ps[:])
                nc.sync.dma_start(out_p[m * 128:(m + 1) * 128, :], osb[:])

        def body():
            proj_qk(qT, wq, bq_sb, QT)
            proj_qk(kT, wk, bk_sb, KT)
            proj_v(vT, wv)
            for h in range(hpc):
                for j in range(NJ):
                    attn_block(h, j)
            out_proj()

        for _ in range(rep):
            body()

    _split_waits(nc)
    return nc


def shard_inputs(query, key, value, Wq, bq, Wk, bk, Wv, bv, Wo, bo):
    """Full inputs -> per-core in_maps (host-side transpose/cast/slice)."""
    in_maps = []
    for c in range(N_CORES):
        b = c // CORES_PER_B
        g = c % CORES_PER_B
        sl = slice(g * HPC * HD, (g + 1) * HPC * HD)
        in_maps.append({
            "qT": np.ascontiguousarray(query[b].T).astype(_bf16),
            "kT": np.ascontiguousarray(key[b].T).astype(_bf16),
            "vT": np.ascontiguousarray(value[b].T).astype(_bf16),
            "wq": np.ascontiguousarray(Wq[:, sl]).astype(_bf16),
            "wk": np.ascontiguousarray(Wk[:, sl]).astype(_bf16),
            "wv": np.ascontiguousarray(Wv[:, sl]).astype(_bf16),
            "wo": np.ascontiguousarray(Wo[sl, :]).astype(_bf16),
            "bq": np.asarray(bq[sl], np.float32).reshape(-1, 1).copy(),
            "bk": np.asarray(bk[sl], np.float32).reshape(-1, 1).copy(),
            "bv": np.asarray(bv[sl], np.float32).reshape(-1, 1).copy(),
        })
    return in_maps


def assemble_outputs(results, bo):
    """Per-core {out_w, out_p} -> (out [B,S,D], attn_weights [B,H,S,S])."""
    attn = np.empty((B, H, S, S), np.float32)
    out = np.zeros((B, S, D), np.float32)
    for c in range(N_CORES):
        b = c // CORES_PER_B
        g = c % CORES_PER_B
        attn[b, g * HPC:(g + 1) * HPC] = results[c]["out_w"].transpose(0, 2, 1)
        out[b] += results[c]["out_p"]
    out += np.asarray(bo, np.float32)
    return out, attn


_nc_cache = {}


def kernel(query, key, value, Wq, bq, Wk, bk, Wv, bv, Wo, bo):
    from concourse.bass_utils import run_bass_kernel_spmd

    if "nc" not in _nc_cache:
        _nc_cache["nc"] = build_nc()
    nc = _nc_cache["nc"]
    in_maps = shard_inputs(query, key, value, Wq, bq, Wk, bk, Wv, bv, Wo, bo)
    res = run_bass_kernel_spmd(nc, in_maps, core_ids=list(range(N_CORES)))
    return assemble_outputs(res.results, bo)


# revision 16
# speedup vs baseline: 2.6369x; 2.6369x over previous
"""Multi-head attention (B=2, S=2048, D=1024, H=16, HD=64) on 8 TRN2 cores.

Sharding: batch x head-groups -> 8 shards (core c: batch c//4, heads
4*(c%4)..4*(c%4)+3). Each core projects its batch's q/k/v against its head
slice of Wq/Wk/Wv, computes full SxS attention for its 4 heads, and the
partial output projection for its head slice of Wo. Host sums the partials
per batch (+bo) and reassembles attn_weights.

On-device layout is "transposed": activations are kept head-dim-major
([hd, S]) so every matmul contracts over the partition axis:
  scoresT[Sk,Sq] = KT_h.T-tiles @ QT_h     (K=64 per head)
  softmax along Sk (partition axis): row sums via ones-matmul on PE,
  reciprocal + scale on DVE; exp on ACT (scale=1/sqrt(HD) folded in).
  outT[hd,Sq] = V-tiles(lhsT) @ wT-tiles   (K=Sk)
attn_weights are written to DRAM in [Sk,Sq] layout and transposed on host.
Matmul operands are bf16 (PE streams 1 col/cycle; fp32 is 4x slower),
accumulation is fp32 in PSUM.
"""
from contextlib import ExitStack

import numpy as np
import ml_dtypes

import concourse.bass as bass
import concourse.mybir as mybir
import concourse.tile as tile

B, S, D, H, HD = 2, 2048, 1024, 16, 64
N_CORES = 8
CORES_PER_B = N_CORES // B          # 4
HPC = H // CORES_PER_B              # heads per core = 4
SCALE = HD ** -0.5

BF16 = mybir.dt.bfloat16
F32 = mybir.dt.float32
AF = mybir.ActivationFunctionType

_bf16 = ml_dtypes.bfloat16


# --- workaround: this walrus build rejects >1 sync wait per instruction ---
_wsplit_ctr = [0]


def _split_waits(nc):
    n = 0
    for fn in nc.m.functions:
        for bb in fn.blocks:
            lst = bb.instructions
            i = 0
            while i < len(lst):
                inst = lst[i]
                si = inst.sync_info
                waits = list(si.on_wait) if si and si.on_wait else []
                if len(waits) > 1:
                    inst.sync_info = mybir.SyncInfo(
                        on_wait=[waits[-1]], on_update=list(si.on_update or [])
                    )
                    for w in waits[:-1]:
                        _wsplit_ctr[0] += 1
                        nop = mybir.InstNoOp(
                            name=f"Wsplit-{_wsplit_ctr[0]}", ins=[], outs=[]
                        )
                        nop.engine = inst.engine
                        nop.sync_info = mybir.SyncInfo(on_wait=[w], on_update=[])
                        nc.register_instruction(nop)
                        lst.insert(i, nop)
                        i += 1
                        n += 1
                i += 1
    return n


def build_nc(rep: int = 1, s: int = S, d: int = D, hpc: int = HPC):
    """Per-core kernel. rep>1 statically unrolls the body (for timing)."""
    KD = d // 128          # contraction tiles of the projections
    SK = s // 128          # Sk tiles (and Sq tiles of the output projection)
    NJ = s // 512          # Sq blocks of 512
    MQ = hpc * HD // 128   # 128-partition tiles of the per-core head dims
    HC = hpc * HD          # head-dim columns per core

    nc = bass.Bass("TRN2")
    qT = nc.dram_tensor("qT", [d, s], BF16, kind="ExternalInput")
    kT = nc.dram_tensor("kT", [d, s], BF16, kind="ExternalInput")
    vT = nc.dram_tensor("vT", [d, s], BF16, kind="ExternalInput")
    wq = nc.dram_tensor("wq", [d, HC], BF16, kind="ExternalInput")
    wk = nc.dram_tensor("wk", [d, HC], BF16, kind="ExternalInput")
    wv = nc.dram_tensor("wv", [d, HC], BF16, kind="ExternalInput")
    wo = nc.dram_tensor("wo", [HC, d], BF16, kind="ExternalInput")
    bq = nc.dram_tensor("bq", [HC, 1], F32, kind="ExternalInput")
    bk = nc.dram_tensor("bk", [HC, 1], F32, kind="ExternalInput")
    bv = nc.dram_tensor("bv", [HC, 1], F32, kind="ExternalInput")
    out_w = nc.dram_tensor("out_w", [hpc, s, s], BF16, kind="ExternalOutput")
    out_p = nc.dram_tensor("out_p", [s, d], F32, kind="ExternalOutput")

    with tile.TileContext(nc) as tc, ExitStack() as ctx:
        const = ctx.enter_context(tc.tile_pool(name="const", bufs=1))
        persist = ctx.enter_context(tc.tile_pool(name="persist", bufs=1))
        xt_pool = ctx.enter_context(tc.tile_pool(name="xt", bufs=1))
        w_pool = ctx.enter_context(tc.tile_pool(name="w", bufs=2))
        et_pool = ctx.enter_context(tc.tile_pool(name="et", bufs=3))
        rc_pool = ctx.enter_context(tc.tile_pool(name="rc", bufs=2))
        ob_pool = ctx.enter_context(tc.tile_pool(name="ob", bufs=2))
        ps_s = ctx.enter_context(tc.tile_pool(name="ps_s", bufs=2, space="PSUM"))
        ps_w = ctx.enter_context(tc.tile_pool(name="ps_w", bufs=4, space="PSUM"))



        bq_sb = const.tile([128, MQ, 1], F32, tag="bq")
        bk_sb = const.tile([128, MQ, 1], F32, tag="bk")
        bv_sb = const.tile([64, hpc, 1], F32, tag="bv")
        nc.sync.dma_start(bq_sb[:], bq.rearrange("(m p) one -> p m one", p=128))
        nc.sync.dma_start(bk_sb[:], bk.rearrange("(m p) one -> p m one", p=128))
        nc.sync.dma_start(bv_sb[:], bv.rearrange("(h p) one -> p h one", p=64))

        # persistent activations
        QT = [persist.tile([128, s], BF16, tag=f"QT{m}", name=f"QT{m}") for m in range(MQ)]
        KT = [persist.tile([128, s], BF16, tag=f"KT{m}", name=f"KT{m}") for m in range(MQ)]
        # V columns per head plus a ones column (row-sum fold into the AV
        # matmul: out row 64 = sum over Sk of the raw exp weights)
        V = persist.tile([128, SK, 65 * hpc], BF16, tag="V")
        aoT = [persist.tile([64, s], BF16, tag=f"aoT{h}", name=f"aoT{h}") for h in range(hpc)]
        wo_sb = [persist.tile([64, d], BF16, tag=f"wo{h}", name=f"wo{h}") for h in range(hpc)]
        for h in range(hpc):
            nc.sync.dma_start(wo_sb[h][:], wo[h * HD:(h + 1) * HD, :])

        def proj_qk(xdram, wdram, bias_sb, OT):
            xt = xt_pool.tile([128, KD, s], BF16, tag="xt")
            nc.sync.dma_start(xt[:], xdram.rearrange("(k p) s -> p k s", p=128))
            wsb = w_pool.tile([128, KD, HC], BF16, tag="w")
            nc.sync.dma_start(wsb[:], wdram.rearrange("(k p) c -> p k c", p=128))
            for m in range(MQ):
                for n in range(s // 512):
                    ps = ps_w.tile([128, 512], F32, tag="ps")
                    for kk in range(KD):
                        nc.tensor.matmul(
                            ps[:],
                            wsb[:, kk, m * 128:(m + 1) * 128],
                            xt[:, kk, n * 512:(n + 1) * 512],
                            start=(kk == 0), stop=(kk == KD - 1),
                        )
                    nc.vector.tensor_scalar_add(
                        OT[m][:, n * 512:(n + 1) * 512], ps[:], bias_sb[:, m, :]
                    )

        def proj_v(xdram, wdram):
            nc.vector.memset(V[:], 1.0)  # fills the ones columns
            xt = xt_pool.tile([128, KD, s], BF16, tag="xt")
            nc.sync.dma_start(xt[:], xdram.rearrange("(k p) s -> p k s", p=128))
            wsb = w_pool.tile([128, KD, HC], BF16, tag="w")
            nc.sync.dma_start(wsb[:], wdram.rearrange("(k p) c -> p k c", p=128))
            for m in range(SK):
                ps = ps_w.tile([128, HC], F32, tag="ps")
                for kk in range(KD):
                    nc.tensor.matmul(
                        ps[:],
                        xt[:, kk, m * 128:(m + 1) * 128],
                        wsb[:, kk, :],
                        start=(kk == 0), stop=(kk == KD - 1),
                    )
                nc.vector.tensor_copy(
                    V[:, m, :].rearrange("p (h c) -> p h c", c=65)[:, :, 0:64],
                    ps[:],
                )

        def attn_block(h, j):
            mq, off = h // 2, 64 * (h % 2)
            KTh = KT[mq][off:off + 64, :]
            QTh = QT[mq][off:off + 64, :]
            et = et_pool.tile([128, SK, 512], BF16, tag="et")
            for ii in range(SK // 2):
                ps = ps_s.tile([128, 2, 512], F32, tag="ps_s")
                for half in (0, 1):
                    i = 2 * ii + half
                    nc.tensor.matmul(
                        ps[:, half, :],
                        KTh[:, i * 128:(i + 1) * 128],
                        QTh[:, j * 512:(j + 1) * 512],
                        start=True, stop=True,
                    )
                nc.scalar.activation(
                    et[:, 2 * ii:2 * ii + 2, :], ps[:], AF.Exp, scale=SCALE
                )
            # AV on the raw exp; row 64 of the psum accumulates the row sums
            # through the ones column of V
            ps_av = ps_w.tile([128, 512], F32, tag="ps")
            for i in range(SK):
                nc.tensor.matmul(
                    ps_av[0:65, :], V[:, i, 65 * h:65 * h + 65], et[:, i, :],
                    start=(i == 0), stop=(i == SK - 1),
                )
            rcrow = rc_pool.tile([1, 512], BF16, tag="rcrow")
            with nc.allow_low_precision(reason="bf16 softmax normalizer"):
                nc.vector.reciprocal(rcrow[:], ps_av[64:65, :])
            recip = rc_pool.tile([128, 512], BF16, tag="rc")
            nc.gpsimd.partition_broadcast(recip[:], rcrow[:])
            # attn out = uout * recip + bv
            nc.vector.tensor_mul(
                aoT[h][:, j * 512:(j + 1) * 512], ps_av[0:64, :], recip[0:64, :]
            )
            nc.vector.tensor_scalar_add(
                aoT[h][:, j * 512:(j + 1) * 512],
                aoT[h][:, j * 512:(j + 1) * 512], bv_sb[:, h, :]
            )
            HK = SK // 2
            for half in (0, 1):
                for i in range(half * HK, (half + 1) * HK):
                    nc.vector.tensor_mul(et[:, i, :], et[:, i, :], recip[:])
                nc.sync.dma_start(
                    out=out_w[h][half * HK * 128:(half + 1) * HK * 128,
                                 j * 512:(j + 1) * 512]
                    .rearrange("(i p) f -> p i f", p=128),
                    in_=et[:, half * HK:(half + 1) * HK, :],
                )

        def out_proj():
            CH = min(512, d)
            for m in range(SK):
                osb = ob_pool.tile([128, d], F32, tag="ob")
                for n in range(d // CH):
                    ps = ps_w.tile([128, CH], F32, tag="ps")
                    for h in range(hpc):
                        nc.tensor.matmul(
                            ps[:],
                            aoT[h][:, m * 128:(m + 1) * 128],
                            wo_sb[h][:, n * CH:(n + 1) * CH],
                            start=(h == 0), stop=(h == hpc - 1),
                        )
                    nc.vector.tensor_copy(osb[:, n * CH:(n + 1) * CH], ps[:])
                nc.sync.dma_start(out_p[m * 128:(m + 1) * 128, :], osb[:])

        def body():
            proj_qk(qT, wq, bq_sb, QT)
            proj_qk(kT, wk, bk_sb, KT)
            proj_v(vT, wv)
            for h in range(hpc):
                for j in range(NJ):
                    attn_block(h, j)
            out_proj()

        for _ in range(rep):
            body()

    _split_waits(nc)
    return nc


def shard_inputs(query, key, value, Wq, bq, Wk, bk, Wv, bv, Wo, bo):
    """Full inputs -> per-core in_maps (host-side transpose/cast/slice)."""
    in_maps = []
    for c in range(N_CORES):
        b = c // CORES_PER_B
        g = c % CORES_PER_B
        sl = slice(g * HPC * HD, (g + 1) * HPC * HD)
        in_maps.append({
            "qT": np.ascontiguousarray(query[b].T).astype(_bf16),
            "kT": np.ascontiguousarray(key[b].T).astype(_bf16),
            "vT": np.ascontiguousarray(value[b].T).astype(_bf16),
            "wq": np.ascontiguousarray(Wq[:, sl]).astype(_bf16),
            "wk": np.ascontiguousarray(Wk[:, sl]).astype(_bf16),
            "wv": np.ascontiguousarray(Wv[:, sl]).astype(_bf16),
            "wo": np.ascontiguousarray(Wo[sl, :]).astype(_bf16),
            "bq": np.asarray(bq[sl], np.float32).reshape(-1, 1).copy(),
            "bk": np.asarray(bk[sl], np.float32).reshape(-1, 1).copy(),
            "bv": np.asarray(bv[sl], np.float32).reshape(-1, 1).copy(),
        })
    return in_maps


def assemble_outputs(results, bo):
    """Per-core {out_w, out_p} -> (out [B,S,D], attn_weights [B,H,S,S])."""
    attn = np.empty((B, H, S, S), np.float32)
    out = np.zeros((B, S, D), np.float32)
    for c in range(N_CORES):
        b = c // CORES_PER_B
        g = c % CORES_PER_B
        attn[b, g * HPC:(g + 1) * HPC] = \
            results[c]["out_w"].transpose(0, 2, 1).astype(np.float32)
        out[b] += results[c]["out_p"]
    out += np.asarray(bo, np.float32)
    return out, attn


_nc_cache = {}


def kernel(query, key, value, Wq, bq, Wk, bk, Wv, bv, Wo, bo):
    from concourse.bass_utils import run_bass_kernel_spmd

    if "nc" not in _nc_cache:
        _nc_cache["nc"] = build_nc()
    nc = _nc_cache["nc"]
    in_maps = shard_inputs(query, key, value, Wq, bq, Wk, bk, Wv, bv, Wo, bo)
    res = run_bass_kernel_spmd(nc, in_maps, core_ids=list(range(N_CORES)))
    return assemble_outputs(res.results, bo)
